# revision 1
# baseline (speedup 1.0000x reference)
# Bass/Tile Trainium2 kernel for nn_Attention_48816598286380.
#
# Reference computation (B=4, N=512, M=8192, Hq=512, Ck=256, H=8, D=64):
#   q = x @ Wq;  k,v = split(context @ Wkv);  per-head softmax(q k^T / sqrt(D)) v
#   out = attn_out @ Wo + bo
#
# Sharding: 8 cores = 4 batches x 2 head-groups (4 heads each).  Each core
# computes its batch's attention for its 4 heads plus the partial output
# projection over those heads; the host sums the two partial projections per
# batch (pure unshard of a sum-sharded tensor; bo is split half/half so the
# sum carries the full bias).
#
# On-device layout: everything is computed in "transposed" orientation so all
# matmul contractions sit on the partition axis:
#   qT[d, n], kT[d, m] from xT / contextT (host supplies the transposes)
#   scoresT[m, n] = kT(m-tile).T @ qT      (two heads packed via PE row tiling)
#   E = exp(scoresT / 8)  on ScalarE, PSUM -> SBUF, [128, 1024] per instr
#   numerT[d, n] (+ denominator row 64) = v_aug(m-tile).T @ E  accumulated in
#     PSUM, where v_aug = [v | ones], so the softmax denominator falls out of
#     the same matmul.
#   out_partial[n, f] = sum_h (numerT_h / den_h) contracted with Wo rows.
#
# All matmul-feeding tensors are declared float32r (full-rate fp32 path on
# the PE; plain fp32 runs at 1/4 rate; walrus requires producers to declare
# f32r output).  Two passes over m (one per head pair) keep the PSUM
# budget at 8 banks; kT/v production is software-pipelined one chunk ahead of
# the attention loop, and pair 1's kT plus all of v stay resident in SBUF so
# pass 1 needs no DMA or production work.

import numpy as np

B, N, M = 4, 512, 8192
QUERY_DIM, INPUT_DIM = 512, 256
HEADS, DIM_HEAD = 8, 64
ATT_DIM = HEADS * DIM_HEAD  # 512
HPC = 4          # heads per core
N_CORES = 8
# chunk schedule: two small chunks first so the first scores/exp start
# as early as possible, then full-size chunks
CHUNKS = [(0, 512), (512, 512)] + [(m0, 1024) for m0 in range(1024, M, 1024)]
MCHUNK = 1024    # max chunk size (pool slot size)
SCALE = DIM_HEAD ** -0.5

_CACHE = {}


def _build_nc():
    import concourse.bacc as bacc
    import concourse.bass as bass
    import concourse.mybir as mybir
    import concourse.tile as tile

    f32 = mybir.dt.float32
    f32r = mybir.dt.float32r
    EXP = mybir.ActivationFunctionType.Exp

    nc = bacc.Bacc(None, target_bir_lowering=False)

    ct = nc.dram_tensor("ct", [INPUT_DIM, M], f32r, kind="ExternalInput")  # context[b].T
    xt = nc.dram_tensor("xt", [QUERY_DIM, N], f32r, kind="ExternalInput")  # x[b].T
    wq = nc.dram_tensor("wq", [QUERY_DIM, HPC * DIM_HEAD], f32r, kind="ExternalInput")
    wk = nc.dram_tensor("wk", [INPUT_DIM, HPC * DIM_HEAD], f32r, kind="ExternalInput")
    wv = nc.dram_tensor("wv", [INPUT_DIM, HPC * DIM_HEAD], f32r, kind="ExternalInput")
    wo = nc.dram_tensor("wo", [DIM_HEAD, HPC, QUERY_DIM], f32r, kind="ExternalInput")
    bo2 = nc.dram_tensor("bo2", [1, QUERY_DIM], f32, kind="ExternalInput")  # bo / 2
    out = nc.dram_tensor("out", [N, QUERY_DIM], f32, kind="ExternalOutput")

    ct_r = ct[:, :].rearrange("(t p) m -> p t m", p=128)    # [128, 2, M]
    xt_r = xt[:, :].rearrange("(t p) n -> p t n", p=128)    # [128, 4, N]
    wq_r = wq[:, :].rearrange("(t p) d -> p t d", p=128)    # [128, 4, 256]
    wk_r = wk[:, :].rearrange("(t p) d -> p t d", p=128)    # [128, 2, 256]
    wv_r = wv[:, :].rearrange("(t p) d -> p t d", p=128)    # [128, 2, 256]
    out_r = out[:, :].rearrange("(t p) f -> p t f", p=128)  # [128, 4, 512]

    with tile.TileContext(nc) as tc:
        with (
            tc.tile_pool(name="const", bufs=1) as cp,
            tc.tile_pool(name="ctp", bufs=2) as ctp,
            tc.tile_pool(name="ktp", bufs=2) as ktp,
            tc.tile_pool(name="ep", bufs=5) as ep,
            tc.tile_pool(name="scp", bufs=3, space="PSUM") as scp,
            tc.tile_pool(name="accp", bufs=1, space="PSUM") as accp,
        ):
            # ---- constants ----
            xt_sb = cp.tile([128, 4, N], f32r)
            wq_sb = cp.tile([128, 4, HPC * DIM_HEAD], f32r)
            wk_sb = cp.tile([128, 2, HPC * DIM_HEAD], f32r)
            wv_sb = cp.tile([128, 2, HPC * DIM_HEAD], f32r)
            wo_sb = cp.tile([DIM_HEAD, HPC, QUERY_DIM], f32r)
            bo_sb = cp.tile([1, QUERY_DIM], f32)
            bo_bc = cp.tile([128, QUERY_DIM], f32)
            qt_sb = cp.tile([128, 2, N], f32r)
            # v for all 4 heads, all of M, with a ones column per head:
            # [128 (m within tile), m-tile, head, 64 v | 1 one]
            v_full = cp.tile([128, M // 128, HPC, DIM_HEAD + 1], f32r)
            stack_sb = cp.tile([DIM_HEAD, HPC, N], f32r)  # normalized attn outT
            recip_sb = cp.tile([128, 2, N], f32)          # partition 64, per pass
            bcast_sb = cp.tile([DIM_HEAD, 2, N], f32)
            out0_sb = cp.tile([128, 4, QUERY_DIM], f32)   # pair-0 proj + bias
            out_sb = cp.tile([128, 4, QUERY_DIM], f32)

            # prologue DMAs, ordered so the first production chunk and qT can
            # start as early as possible; the rest of the prologue (wv, ones,
            # wo, bias) is emitted after chunk 0's context DMA below.
            nc.sync.dma_start(out=wq_sb[:], in_=wq_r)
            nc.sync.dma_start(out=xt_sb[:], in_=xt_r)
            nc.sync.dma_start(out=wk_sb[:], in_=wk_r)

            # PE warm-up: the HAM clock gate holds the PE at 1.2 GHz until
            # ~3.4 us of sustained activity.  Run throwaway matmuls on a
            # zeroed tile while the prologue DMAs are in flight so qT/kT and
            # the first score tiles run at full clock.
            warm_sb = cp.tile([128, 64], f32)
            nc.vector.memset(warm_sb[:], 0.0)
            warm_ps = accp.tile([128, N], f32, tag="acc0", name="warm_ps")
            for w in range(24):
                nc.tensor.matmul(
                    warm_ps[0:64, 0:64], lhsT=warm_sb[:], rhs=warm_sb[:],
                    start=True, stop=True, skip_group_check=True,
                )

            # kT for pair 1 stays resident; pair 0's kT lives in rotating
            # chunk tiles consumed during pass 0.
            kt_f1 = ktp.tile([128, M], f32r, tag="ktf1", name="ktf1", bufs=1)
            kt_of = {}

            def produce_chunk(mc):
                """DMA chunk mc of contextT; return kT/v production emitters
                (closures) so production interleaves with attention tiles."""
                m0, mlen = CHUNKS[mc]
                ct_t = ctp.tile([128, 2, MCHUNK], f32r, tag="ct", name=f"ct{mc}")
                ct_dma = nc.sync.dma_start(
                    out=ct_t[:, :, 0:mlen], in_=ct_r[:, :, m0:m0 + mlen]
                )
                if mc >= 1:
                    # keep the small prologue DMAs ahead of the chunk stream
                    # on the SP queue
                    for d in late_dmas:
                        tile.add_dep_helper(ct_dma.ins, d.ins, sync=False,
                                            reason="prologue before ct stream")
                kt_t = ktp.tile([128, MCHUNK], f32r, tag="kt", name=f"kt{mc}")
                for mi in range(m0 // 128, (m0 + mlen) // 128):
                    kt_of[mi] = (kt_t, mi * 128 - m0)
                halves = mlen // 512

                def kt_group(pp):
                    def go():
                        kt_ps = scp.tile([128, 1024], f32, tag="sc",
                                         name=f"ktps{pp}{mc}")
                        for h2 in range(halves):
                            for t in range(2):
                                nc.tensor.matmul(
                                    kt_ps[:, h2 * 512:(h2 + 1) * 512],
                                    lhsT=wk_sb[:, t, pp * 128:(pp + 1) * 128],
                                    rhs=ct_t[:, t, h2 * 512:(h2 + 1) * 512],
                                    start=(t == 0), stop=(t == 1),
                                    skip_group_check=True,
                                )
                        dst = (kt_t[:, 0:mlen] if pp == 0 else
                               kt_f1[:, m0:m0 + mlen])
                        nc.vector.tensor_copy(dst, kt_ps[:, 0:mlen])
                    return go

                def v_group(s4):
                    def go():
                        v_ps = scp.tile([128, 1024], f32, tag="sc",
                                        name=f"vps{mc}{s4}")
                        for q in range(4):
                            s = s4 * 4 + q
                            for t in range(2):
                                nc.tensor.matmul(
                                    v_ps[:, q * 256:(q + 1) * 256],
                                    lhsT=ct_t[:, t, s * 128:(s + 1) * 128],
                                    rhs=wv_sb[:, t, :],
                                    start=(t == 0), stop=(t == 1),
                                    skip_group_check=True,
                                )
                        nc.vector.tensor_copy(
                            v_full[:, m0 // 128 + s4 * 4:
                                   m0 // 128 + s4 * 4 + 4, :, 0:DIM_HEAD],
                            v_ps[:].rearrange("p (s h d) -> p s h d", s=4, h=HPC),
                        )
                    return go

                # order: pair-0 kT first (needed immediately), v next (needed
                # by AV shortly after), pair-1 kT last (pass 1 only)
                if mlen == 512:
                    return [kt_group(0), v_group(0), kt_group(1)]
                return [kt_group(0), v_group(0), v_group(1), kt_group(1)]

            def qk_exp(p, mi):
                sc = scp.tile([128, 1024], f32, tag="sc", name=f"sc{p}{mi}")
                if p == 0:
                    ks, off = kt_of[mi]
                else:
                    ks, off = kt_f1, mi * 128
                ks = ks[:, off:off + 128]
                # two heads in one PE pass via row tiling
                nc.tensor.matmul(sc[:, 0:512], lhsT=ks[0:64, :],
                                 rhs=qt_sb[0:64, p, :], start=True, stop=True)
                nc.tensor.matmul(sc[:, 512:1024], lhsT=ks[64:128, :],
                                 rhs=qt_sb[64:128, p, :], start=True, stop=True)
                e_t = ep.tile([128, 1024], f32r, tag="e", name=f"e{p}{mi}")
                nc.scalar.activation(e_t[:], sc[:], EXP, scale=SCALE)
                return e_t

            def av(p, mi, e_t, acc):
                for h2 in range(2):
                    nc.tensor.matmul(
                        acc[h2][0:DIM_HEAD + 1, :],
                        lhsT=v_full[:, mi, 2 * p + h2, :],
                        rhs=e_t[:, h2 * 512:(h2 + 1) * 512],
                        start=(mi == 0), stop=(mi == M // 128 - 1),
                        skip_group_check=True,
                    )

            def attention_tile(p, mi, acc):
                av(p, mi, qk_exp(p, mi), acc)

            def pass_tail(p, acc):
                """normalize numerators by the ones-row denominator.  The
                reciprocal lands on partition 64 (DVE is lane-locked); a K=1
                matmul with both operands based at partition 64 broadcasts it
                to partitions 0-63 (same row-tiling path QK uses), much
                faster than the shift-DMA + gpsimd partition_broadcast."""
                bc_ps = scp.tile([128, 1024], f32, tag="sc", name=f"bc{p}")
                for h2 in range(2):
                    nc.vector.reciprocal(
                        recip_sb[DIM_HEAD:DIM_HEAD + 1, h2, :],
                        acc[h2][DIM_HEAD:DIM_HEAD + 1, :],
                    )
                    nc.tensor.matmul(
                        bc_ps[0:DIM_HEAD, h2 * 512:(h2 + 1) * 512],
                        lhsT=ones64_sb[DIM_HEAD:DIM_HEAD + 1, :],
                        rhs=recip_sb[DIM_HEAD:DIM_HEAD + 1, h2, :],
                        start=True, stop=True, skip_group_check=True,
                    )
                    nc.vector.tensor_copy(
                        bcast_sb[:, h2, :],
                        bc_ps[0:DIM_HEAD, h2 * 512:(h2 + 1) * 512],
                    )
                    nc.vector.tensor_mul(
                        stack_sb[:, 2 * p + h2, :], acc[h2][0:DIM_HEAD, :],
                        bcast_sb[:, h2, :]
                    )

            # chunk-0 context DMA goes out right behind the qT weights
            chunk0 = produce_chunk(0)

            # late prologue (not needed until mid-kernel)
            late_dmas = []
            late_dmas.append(nc.sync.dma_start(out=wv_sb[:], in_=wv_r))
            late_dmas.append(nc.sync.dma_start(out=wo_sb[:], in_=wo[:, :, :]))
            late_dmas.append(nc.sync.dma_start(out=bo_sb[:], in_=bo2[:, :]))
            # ones column of v_aug: memset a [128, 1] column, then one
            # broadcast-copy into the strided ones slots (rounds to f32r)
            ones_col = cp.tile([128, 1], f32)
            nc.vector.memset(ones_col[:], 1.0)
            ones64_sb = cp.tile([128, DIM_HEAD], f32)
            nc.vector.memset(ones64_sb[:], 1.0)
            _oc, _vdst = bass.broadcast_tensor_aps(
                ones_col[:, :], v_full[:, :, :, DIM_HEAD].rearrange(
                    "p s h -> p (s h)")[:, None, :].rearrange("p o q -> p (o q)")
            )
            nc.vector.tensor_copy(_vdst, _oc)
            nc.gpsimd.partition_broadcast(bo_bc[:], bo_sb[0:1, :])

            # qT per head-pair p: [128, N]; rows 0-63 head 2p, 64-127 head 2p+1
            q_ps = scp.tile([128, 1024], f32, tag="sc", name="q_ps")
            for p in range(2):
                for t in range(4):
                    nc.tensor.matmul(
                        q_ps[:, p * 512:(p + 1) * 512],
                        lhsT=wq_sb[:, t, p * 128:(p + 1) * 128],
                        rhs=xt_sb[:, t, :],
                        start=(t == 0), stop=(t == 3),
                        skip_group_check=True,
                    )
            nc.vector.tensor_copy(
                qt_sb[:, :, :], q_ps[:].rearrange("p (a n) -> p a n", a=2))

            # ---- pass 0 (heads 0,1), production pipelined one chunk ahead --
            acc0 = [accp.tile([128, N], f32, tag=f"acc{h2}", name=f"a0{h2}")
                    for h2 in range(2)]
            prefetch = {}
            for step in range(len(CHUNKS) + 1):
                prod = (chunk0 if step == 0 else produce_chunk(step)) \
                    if step < len(CHUNKS) else []
                if step >= 1:
                    pm0, pmlen = CHUNKS[step - 1]
                    atts = list(range(pm0 // 128, (pm0 + pmlen) // 128))
                else:
                    atts = []
                for i in range(max(2 * len(prod), len(atts))):
                    if i < len(atts):
                        attention_tile(0, atts[i], acc0)
                    if i % 2 == 0 and i // 2 < len(prod):
                        prod[i // 2]()
            pass_tail(0, acc0)

            # partial projection for pair 0 (+ bias) overlaps pass 1
            def proj_pair0():
                for g in range(2):
                    pr0 = scp.tile([128, 1024], f32, tag="sc", name=f"pr0{g}")
                    for j in range(2):
                        nt = g * 2 + j
                        for h in range(2):
                            nc.tensor.matmul(
                                pr0[:, j * 512:(j + 1) * 512],
                                lhsT=stack_sb[:, h, nt * 128:(nt + 1) * 128],
                                rhs=wo_sb[:, h, :],
                                start=(h == 0), stop=(h == 1),
                                skip_group_check=True,
                            )
                    for j in range(2):
                        nt = g * 2 + j
                        nc.vector.tensor_add(
                            out0_sb[:, nt, :], pr0[:, j * 512:(j + 1) * 512],
                            bo_bc[:])

            # ---- pass 1 (heads 2,3): pure attention from resident kT/v ----
            acc1 = [accp.tile([128, N], f32, tag=f"acc{h2}", name=f"a1{h2}")
                    for h2 in range(2)]
            for mi in range(M // 128):
                if mi in prefetch:
                    av(1, mi, prefetch.pop(mi), acc1)
                else:
                    attention_tile(1, mi, acc1)
                if mi == 8:
                    proj_pair0()
            pass_tail(1, acc1)

            # ---- pair-1 projection + combine + store ----
            for g in range(2):
                pr = scp.tile([128, 1024], f32, tag="sc", name=f"pr{g}")
                for j in range(2):
                    nt = g * 2 + j
                    for h in range(2, 4):
                        nc.tensor.matmul(
                            pr[:, j * 512:(j + 1) * 512],
                            lhsT=stack_sb[:, h, nt * 128:(nt + 1) * 128],
                            rhs=wo_sb[:, h, :],
                            start=(h == 2), stop=(h == 3),
                            skip_group_check=True,
                        )
                for j in range(2):
                    nt = g * 2 + j
                    nc.vector.tensor_add(
                        out_sb[:, nt, :], pr[:, j * 512:(j + 1) * 512],
                        out0_sb[:, nt, :])
                    nc.sync.dma_start(out=out_r[:, nt, :], in_=out_sb[:, nt, :])

    nc.compile()
    return nc


def _get_nc():
    if "nc" not in _CACHE:
        _CACHE["nc"] = _build_nc()
    return _CACHE["nc"]


def _make_in_maps(x, context, Wq, Wkv, Wo, bo):
    x = np.asarray(x, dtype=np.float32)
    context = np.asarray(context, dtype=np.float32)
    Wq = np.asarray(Wq, dtype=np.float32)
    Wkv = np.asarray(Wkv, dtype=np.float32)
    Wo = np.asarray(Wo, dtype=np.float32)
    bo = np.asarray(bo, dtype=np.float32)

    Wk = Wkv[:, :ATT_DIM]
    Wv = Wkv[:, ATT_DIM:]
    bo2 = np.ascontiguousarray((bo / 2.0)[None, :])

    in_maps = []
    for c in range(N_CORES):
        b, g = divmod(c, 2)
        hs = g * HPC * DIM_HEAD           # column offset of this core's heads
        he = hs + HPC * DIM_HEAD
        wo_core = Wo[hs:he, :].reshape(HPC, DIM_HEAD, QUERY_DIM)
        in_maps.append({
            "ct": np.ascontiguousarray(context[b].T),
            "xt": np.ascontiguousarray(x[b].T),
            "wq": np.ascontiguousarray(Wq[:, hs:he]),
            "wk": np.ascontiguousarray(Wk[:, hs:he]),
            "wv": np.ascontiguousarray(Wv[:, hs:he]),
            "wo": np.ascontiguousarray(wo_core.transpose(1, 0, 2)),
            "bo2": bo2,
        })
    return in_maps


def run(inputs, trace=False, **spmd_kwargs):
    """Run the kernel; returns (full_output [B,N,QUERY_DIM], BassKernelResults)."""
    from concourse.bass_utils import run_bass_kernel_spmd

    nc = _get_nc()
    in_maps = _make_in_maps(**inputs)
    res = run_bass_kernel_spmd(
        nc, in_maps, core_ids=list(range(N_CORES)), trace=trace, **spmd_kwargs
    )
    outs = [r["out"] for r in res.results]
    full = np.empty((B, N, QUERY_DIM), dtype=np.float32)
    for b in range(B):
        full[b] = outs[2 * b] + outs[2 * b + 1]
    return full, res


def kernel(**inputs) -> np.ndarray:
    full, _ = run(inputs, trace=False)
    return full



# revision 3
# speedup vs baseline: 1.1213x; 1.1213x over previous
# Bass/Tile Trainium2 kernel for nn_Attention_48816598286380.
#
# Reference computation (B=4, N=512, M=8192, Hq=512, Ck=256, H=8, D=64):
#   q = x @ Wq;  k,v = split(context @ Wkv);  per-head softmax(q k^T / sqrt(D)) v
#   out = attn_out @ Wo + bo
#
# Sharding: 8 cores = 4 batches x 2 head-groups (4 heads each).  Each core
# computes its batch's attention for its 4 heads plus the partial output
# projection over those heads; the host sums the two partial projections per
# batch (bo is split half/half so the sum carries the full bias).
#
# Numerics: Q/K/V and the exp'd scores E are fp8e4 so the two big matmuls
# run in dual-row fp8 mode (2 k-tiles per pass, 0.5 PE cycles/row):
#   scoresT[m, n] = kT.T @ (q8 + q_residual8): the dual-row second k-tile
#     carries a quantized fp8 residual of qT (k-tile dim stride-0 on kT), so
#     only kT's quantization error survives -> rel err ~1.1e-2.
#   E = exp(s/8 - 2) in fp8e4 (softmax shift keeps E <= ~35 << 240), on two
#     engines in parallel: Act exp (PSUM->SBUF, scale+bias) and gpsimd pow
#     c^(s-16), c = e^(1/8) (vpowf ucode; needs a DVE/Act staging copy of
#     s-16 to SBUF since gpsimd cannot read PSUM).
#   numerT[d, n] + denominator row 64 = v_aug.T @ E with TWO m-tiles per
#     instruction (the dual-row k-tile pair; v rows padded to 68 for the
#     ISA's 16-element dual-fp8 weight stride rule).
# kT/qT/v production and the output projection stay f32r (exact inputs).
#
# Engine budget per core (cost model): exp is 128 x [128,1024] tiles split
# Act/Pool; PSUM->SBUF casts split Act/DVE; PE ~80us under the ~95us
# Act/DVE/Pool balance.  PSUM: 2x2-bank score slots (QK->exp), 2x1-bank
# production slots (kT/v/proj -> cast), 2 accumulator banks.

import numpy as np

B, N, M = 4, 512, 8192
QUERY_DIM, INPUT_DIM = 512, 256
HEADS, DIM_HEAD = 8, 64
ATT_DIM = HEADS * DIM_HEAD  # 512
HPC = 4          # heads per core
N_CORES = 8
DH_PAD = 68      # v row pitch: 64 d + ones + 3 pad (16-elem alignment)
CHUNKS = [(0, 512), (512, 512)] + [(m0, 1024) for m0 in range(1024, M, 1024)]
MCHUNK = 1024
SCALE = DIM_HEAD ** -0.5   # 0.125
EBIAS = -2.0               # softmax shift (in exp-arg units)
SBIAS = EBIAS / SCALE      # shift in score units (-16)

# exp engine split: tile index within a pass -> True = Pool pow path
POOL0 = [(i % 2) == 0 for i in range(64)]            # pass 0: 32 on Pool
POOL1 = [(i % 5) in (0, 2) for i in range(64)]       # pass 1: 26 on Pool
# production-cast engine: index within chunk cycle -> True = Act copy
# (kt half copies and v group copies, pass 0 only)
ACT_COPY = [True, True, True, False, True, True, True, False]

_CACHE = {}


def _build_nc():
    import concourse.bacc as bacc
    import concourse.bass as bass
    import concourse.mybir as mybir
    import concourse.tile as tile

    f32 = mybir.dt.float32
    f32r = mybir.dt.float32r
    fp8 = mybir.dt.float8e4
    EXP = mybir.ActivationFunctionType.Exp
    CPY = mybir.ActivationFunctionType.Copy
    POW = mybir.AluOpType.pow
    DR = mybir.MatmulPerfMode.DoubleRow

    def bc0(ap, ins_dims):
        """Insert stride-0 free dims into an AP after the partition dim."""
        return bass.AP(
            tensor=ap.tensor,
            offset=ap.offset,
            ap=[ap.ap[0]] + [[0, n] for n in ins_dims] + list(ap.ap[1:]),
        )

    nc = bacc.Bacc(None, target_bir_lowering=False)

    ct = nc.dram_tensor("ct", [INPUT_DIM, M], f32r, kind="ExternalInput")
    xt = nc.dram_tensor("xt", [QUERY_DIM, N], f32r, kind="ExternalInput")
    wq = nc.dram_tensor("wq", [QUERY_DIM, HPC * DIM_HEAD], f32r, kind="ExternalInput")
    wk = nc.dram_tensor("wk", [INPUT_DIM, HPC * DIM_HEAD], f32r, kind="ExternalInput")
    wv = nc.dram_tensor("wv", [INPUT_DIM, HPC * DIM_HEAD], f32r, kind="ExternalInput")
    wo = nc.dram_tensor("wo", [DIM_HEAD, HPC, QUERY_DIM], f32r, kind="ExternalInput")
    bobc = nc.dram_tensor("bobc", [128, QUERY_DIM], f32, kind="ExternalInput")
    out = nc.dram_tensor("out", [N, QUERY_DIM], f32, kind="ExternalOutput")

    ct_r = ct[:, :].rearrange("(t p) m -> p t m", p=128)
    xt_r = xt[:, :].rearrange("(t p) n -> p t n", p=128)
    wq_r = wq[:, :].rearrange("(t p) d -> p t d", p=128)
    wk_r = wk[:, :].rearrange("(t p) d -> p t d", p=128)
    wv_r = wv[:, :].rearrange("(t p) d -> p t d", p=128)
    out_r = out[:, :].rearrange("(t p) f -> p t f", p=128)

    with tile.TileContext(nc) as tc:
        with (
            tc.tile_pool(name="const", bufs=1) as cp,
            tc.tile_pool(name="ctp", bufs=2) as ctp,
            tc.tile_pool(name="ktp", bufs=2) as ktp,
            tc.tile_pool(name="ep", bufs=6) as ep,
            tc.tile_pool(name="stp", bufs=4) as stp,
            tc.tile_pool(name="scp", bufs=2, space="PSUM") as scp,
            tc.tile_pool(name="prp", bufs=2, space="PSUM") as prp,
            tc.tile_pool(name="accp", bufs=1, space="PSUM") as accp,
        ):
            # ---- constants ----
            xt_sb = cp.tile([128, 4, N], f32r)
            wq_sb = cp.tile([128, 4, HPC * DIM_HEAD], f32r)
            wk_sb = cp.tile([128, 2, HPC * DIM_HEAD], f32r)
            wv_sb = cp.tile([128, 2, HPC * DIM_HEAD], f32r)
            wo_sb = cp.tile([DIM_HEAD, HPC, QUERY_DIM], f32r)
            bo_bc = cp.tile([128, QUERY_DIM], f32)
            # qT fp8: [d(2 heads x 64), pass, ktile, n]; ktile1 = fp8 residual
            qt8 = cp.tile([128, 2, 2, N], fp8)
            # v fp8: [m-in-tile, m-tile, head, 68]; col 64 ones, 65-67 pad
            v8 = cp.tile([128, M // 128, HPC, DH_PAD], fp8)
            kt8_f1 = ktp.tile([128, M], fp8, tag="ktf1", name="ktf1", bufs=1)
            stack_sb = cp.tile([DIM_HEAD, HPC, N], f32r)
            recip_sb = cp.tile([128, 2, N], f32)
            bcast_sb = cp.tile([DIM_HEAD, 2, N], f32)
            ones64_sb = cp.tile([128, DIM_HEAD], f32)
            cexp_sb = cp.tile([128, 1], f32)
            ebias_sb = cp.tile([128, 1], f32)
            out0_sb = cp.tile([128, 4, QUERY_DIM], f32)
            out_sb = cp.tile([128, 4, QUERY_DIM], f32)

            nc.sync.dma_start(out=wq_sb[:], in_=wq_r)
            nc.sync.dma_start(out=xt_sb[:], in_=xt_r)
            nc.sync.dma_start(out=wk_sb[:], in_=wk_r)

            # PE warm-up (p-state ramp: ~3.4us of activity -> full clock)
            warm_sb = cp.tile([128, 64], f32)
            nc.vector.memset(warm_sb[:], 0.0)
            warm_ps = accp.tile([128, N], f32, tag="acc0", name="warm_ps")
            for w in range(24):
                nc.tensor.matmul(
                    warm_ps[0:64, 0:64], lhsT=warm_sb[:], rhs=warm_sb[:],
                    start=True, stop=True, skip_group_check=True,
                )

            kt_of = {}

            def cast_copy(use_act, dst, src):
                if use_act:
                    nc.scalar.activation(dst, src, CPY, scale=1.0, bias=0.0)
                else:
                    nc.vector.tensor_copy(dst, src)

            def produce_chunk(mc):
                """DMA chunk mc of contextT; return production emitters at
                [128,512] granularity (one prp PSUM slot each)."""
                m0, mlen = CHUNKS[mc]
                ct_t = ctp.tile([128, 2, MCHUNK], f32r, tag="ct", name=f"ct{mc}")
                ct_dma = nc.sync.dma_start(
                    out=ct_t[:, :, 0:mlen], in_=ct_r[:, :, m0:m0 + mlen]
                )
                if mc >= 1:
                    for d in late_dmas:
                        tile.add_dep_helper(ct_dma.ins, d.ins, sync=False,
                                            reason="prologue before ct stream")
                kt_t = ktp.tile([128, MCHUNK], fp8, tag="kt", name=f"kt{mc}")
                for mi in range(m0 // 128, (m0 + mlen) // 128):
                    kt_of[mi] = (kt_t, mi * 128 - m0)
                halves = mlen // 512

                def kt_half(pp, h2, eng):
                    def go():
                        kt_ps = prp.tile([128, 512], f32, tag="pr",
                                         name=f"ktps{pp}{mc}{h2}")
                        for t in range(2):
                            nc.tensor.matmul(
                                kt_ps[:, :],
                                lhsT=wk_sb[:, t, pp * 128:(pp + 1) * 128],
                                rhs=ct_t[:, t, h2 * 512:(h2 + 1) * 512],
                                start=(t == 0), stop=(t == 1),
                                skip_group_check=True,
                            )
                        o = h2 * 512
                        dst = (kt_t[:, o:o + 512] if pp == 0 else
                               kt8_f1[:, m0 + o:m0 + o + 512])
                        cast_copy(eng, dst, kt_ps[:, :])
                    return go

                def v_pair(s2, eng):
                    def go():
                        v_ps = prp.tile([128, 512], f32, tag="pr",
                                        name=f"vps{mc}{s2}")
                        for q in range(2):
                            s = s2 * 2 + q
                            for t in range(2):
                                nc.tensor.matmul(
                                    v_ps[:, q * 256:(q + 1) * 256],
                                    lhsT=ct_t[:, t, s * 128:(s + 1) * 128],
                                    rhs=wv_sb[:, t, :],
                                    start=(t == 0), stop=(t == 1),
                                    skip_group_check=True,
                                )
                        mt0 = m0 // 128 + s2 * 2
                        cast_copy(
                            eng,
                            v8[:, mt0:mt0 + 2, :, 0:DIM_HEAD],
                            v_ps[:].rearrange("p (s h d) -> p s h d", s=2,
                                              h=HPC),
                        )
                    return go

                groups = []
                ci = 0
                for h2 in range(halves):
                    groups.append(kt_half(0, h2, ACT_COPY[ci % 8])); ci += 1
                for s2 in range(halves * 2):
                    groups.append(v_pair(s2, ACT_COPY[ci % 8])); ci += 1
                for h2 in range(halves):
                    groups.append(kt_half(1, h2, ACT_COPY[ci % 8])); ci += 1
                return groups

            def qk(p, mi, sc):
                if p == 0:
                    ks, off = kt_of[mi]
                else:
                    ks, off = kt8_f1, mi * 128
                for h2 in range(2):
                    lhs = ks[h2 * 64:(h2 + 1) * 64, off:off + 128]
                    nc.tensor.matmul(
                        sc[:, h2 * 512:(h2 + 1) * 512],
                        lhsT=bc0(lhs, [2]),
                        rhs=qt8[h2 * 64:(h2 + 1) * 64, p, :, :],
                        start=True, stop=True, perf_mode=DR,
                    )

            def exp_tile(sc, e_t, t, use_pool):
                dst = e_t[:, :, t, :]
                if use_pool:
                    st = stp.tile([128, 2, 512], f32, tag="st", name=f"st{t}")
                    nc.vector.tensor_scalar_add(
                        st[:, :, :], sc[:].rearrange("p (h n) -> p h n", h=2),
                        SBIAS)
                    nc.gpsimd.tensor_tensor(
                        dst, bc0(cexp_sb[:, :1], [2, 512]).opt({0}),
                        st[:, :, :], POW)
                else:
                    nc.scalar.activation(
                        dst, sc[:].rearrange("p (h n) -> p h n", h=2),
                        EXP, scale=SCALE, bias=ebias_sb[:, :])

            def av(p, pair, e_t, acc):
                for h2 in range(2):
                    nc.tensor.matmul(
                        acc[h2][0:DH_PAD, :],
                        lhsT=v8[:, 2 * pair:2 * pair + 2, 2 * p + h2, :],
                        rhs=e_t[:, h2, :, :],
                        start=(pair == 0), stop=(pair == M // 256 - 1),
                        perf_mode=DR, skip_group_check=True,
                    )

            def pass_tail(p, acc):
                bc_ps = scp.tile([128, 1024], f32, tag="sc", name=f"bc{p}")
                for h2 in range(2):
                    nc.vector.reciprocal(
                        recip_sb[DIM_HEAD:DIM_HEAD + 1, h2, :],
                        acc[h2][DIM_HEAD:DIM_HEAD + 1, :],
                    )
                    nc.tensor.matmul(
                        bc_ps[0:DIM_HEAD, h2 * 512:(h2 + 1) * 512],
                        lhsT=ones64_sb[DIM_HEAD:DIM_HEAD + 1, :],
                        rhs=recip_sb[DIM_HEAD:DIM_HEAD + 1, h2, :],
                        start=True, stop=True, skip_group_check=True,
                    )
                    nc.vector.tensor_copy(
                        bcast_sb[:, h2, :],
                        bc_ps[0:DIM_HEAD, h2 * 512:(h2 + 1) * 512],
                    )
                    nc.vector.tensor_mul(
                        stack_sb[:, 2 * p + h2, :], acc[h2][0:DIM_HEAD, :],
                        bcast_sb[:, h2, :]
                    )

            chunk0 = produce_chunk(0)

            late_dmas = []
            late_dmas.append(nc.sync.dma_start(out=wv_sb[:], in_=wv_r))
            late_dmas.append(nc.sync.dma_start(out=wo_sb[:], in_=wo[:, :, :]))
            late_dmas.append(nc.sync.dma_start(out=bo_bc[:], in_=bobc[:, :]))
            nc.vector.memset(ones64_sb[:], 1.0)
            nc.vector.memset(cexp_sb[:], float(np.exp(SCALE)))
            nc.vector.memset(ebias_sb[:], EBIAS)
            nc.gpsimd.memset(v8[:, :, :, DIM_HEAD:DH_PAD], 1.0)

            # qT: rows 0-63 head 2p, 64-127 head 2p+1; fp8 + fp8 residual
            q_ps = scp.tile([128, 1024], f32, tag="sc", name="q_ps")
            for p in range(2):
                for t in range(4):
                    nc.tensor.matmul(
                        q_ps[:, p * 512:(p + 1) * 512],
                        lhsT=wq_sb[:, t, p * 128:(p + 1) * 128],
                        rhs=xt_sb[:, t, :],
                        start=(t == 0), stop=(t == 3),
                        skip_group_check=True,
                    )
            q_r = q_ps[:].rearrange("p (a n) -> p a n", a=2)
            nc.vector.tensor_copy(qt8[:, :, 0, :], q_r)
            nc.vector.tensor_sub(qt8[:, :, 1, :], q_r, qt8[:, :, 0, :])

            # ---- pass 0 (heads 0,1), production interleaved ----
            acc0 = [accp.tile([128, N], f32, tag=f"acc{h2}", name=f"a0{h2}")
                    for h2 in range(2)]

            def attention_tile(p, mi, acc, e_state, pool_map):
                pair, t = mi // 2, mi % 2
                sc = scp.tile([128, 1024], f32, tag="sc", name=f"sc{p}{mi}")
                qk(p, mi, sc)
                if t == 0:
                    e_state["tile"] = ep.tile([128, 2, 2, 512], fp8, tag="e",
                                              name=f"e{p}{pair}")
                exp_tile(sc, e_state["tile"], t, pool_map[mi])
                if t == 1:
                    av(p, pair, e_state["tile"], acc)

            e_state = {}
            for step in range(len(CHUNKS) + 1):
                prod = (chunk0 if step == 0 else produce_chunk(step)) \
                    if step < len(CHUNKS) else []
                if step >= 1:
                    pm0, pmlen = CHUNKS[step - 1]
                    atts = list(range(pm0 // 128, (pm0 + pmlen) // 128))
                else:
                    atts = []
                n = max(len(prod), len(atts))
                for i in range(n):
                    if i < len(atts):
                        attention_tile(0, atts[i], acc0, e_state, POOL0)
                    if i < len(prod):
                        prod[i]()
            pass_tail(0, acc0)

            # partial projection for pair 0 (+ bias) overlaps pass 1;
            # uses the production PSUM slots (free during pass 1)
            def proj(hs, dst_sb, add_sb):
                for nt in range(4):
                    pr = prp.tile([128, 512], f32, tag="pr", name=f"pj{hs}{nt}")
                    for h in range(hs, hs + 2):
                        nc.tensor.matmul(
                            pr[:, :],
                            lhsT=stack_sb[:, h, nt * 128:(nt + 1) * 128],
                            rhs=wo_sb[:, h, :],
                            start=(h == hs), stop=(h == hs + 1),
                            skip_group_check=True,
                        )
                    nc.vector.tensor_add(dst_sb[:, nt, :], pr[:, :], add_sb
                                         if add_sb is not None
                                         else bo_bc[:])

            def proj_pair0():
                for nt in range(4):
                    pr = prp.tile([128, 512], f32, tag="pr", name=f"pj0{nt}")
                    for h in range(2):
                        nc.tensor.matmul(
                            pr[:, :],
                            lhsT=stack_sb[:, h, nt * 128:(nt + 1) * 128],
                            rhs=wo_sb[:, h, :],
                            start=(h == 0), stop=(h == 1),
                            skip_group_check=True,
                        )
                    nc.vector.tensor_add(out0_sb[:, nt, :], pr[:, :], bo_bc[:])

            # ---- pass 1 (heads 2,3): pure attention from resident kT/v ----
            acc1 = [accp.tile([128, N], f32, tag=f"acc{h2}", name=f"a1{h2}")
                    for h2 in range(2)]
            e_state1 = {}
            for mi in range(M // 128):
                attention_tile(1, mi, acc1, e_state1, POOL1)
                if mi == 8:
                    proj_pair0()
            pass_tail(1, acc1)

            # ---- pair-1 projection + combine + store ----
            for nt in range(4):
                pr = prp.tile([128, 512], f32, tag="pr", name=f"pj1{nt}")
                for h in range(2, 4):
                    nc.tensor.matmul(
                        pr[:, :],
                        lhsT=stack_sb[:, h, nt * 128:(nt + 1) * 128],
                        rhs=wo_sb[:, h, :],
                        start=(h == 2), stop=(h == 3),
                        skip_group_check=True,
                    )
                nc.vector.tensor_add(out_sb[:, nt, :], pr[:, :],
                                     out0_sb[:, nt, :])
                nc.sync.dma_start(out=out_r[:, nt, :], in_=out_sb[:, nt, :])

    nc.compile()
    return nc


def _get_nc():
    if "nc" not in _CACHE:
        _CACHE["nc"] = _build_nc()
    return _CACHE["nc"]


def _make_in_maps(x, context, Wq, Wkv, Wo, bo):
    x = np.asarray(x, dtype=np.float32)
    context = np.asarray(context, dtype=np.float32)
    Wq = np.asarray(Wq, dtype=np.float32)
    Wkv = np.asarray(Wkv, dtype=np.float32)
    Wo = np.asarray(Wo, dtype=np.float32)
    bo = np.asarray(bo, dtype=np.float32)

    Wk = Wkv[:, :ATT_DIM]
    Wv = Wkv[:, ATT_DIM:]
    bobc = np.ascontiguousarray(
        np.broadcast_to((bo / 2.0)[None, :], (128, QUERY_DIM)))

    in_maps = []
    for c in range(N_CORES):
        b, g = divmod(c, 2)
        hs = g * HPC * DIM_HEAD
        he = hs + HPC * DIM_HEAD
        wo_core = Wo[hs:he, :].reshape(HPC, DIM_HEAD, QUERY_DIM)
        in_maps.append({
            "ct": np.ascontiguousarray(context[b].T),
            "xt": np.ascontiguousarray(x[b].T),
            "wq": np.ascontiguousarray(Wq[:, hs:he]),
            "wk": np.ascontiguousarray(Wk[:, hs:he]),
            "wv": np.ascontiguousarray(Wv[:, hs:he]),
            "wo": np.ascontiguousarray(wo_core.transpose(1, 0, 2)),
            "bobc": bobc,
        })
    return in_maps


def run(inputs, trace=False, **spmd_kwargs):
    """Run the kernel; returns (full_output [B,N,QUERY_DIM], BassKernelResults)."""
    from concourse.bass_utils import run_bass_kernel_spmd

    nc = _get_nc()
    in_maps = _make_in_maps(**inputs)
    res = run_bass_kernel_spmd(
        nc, in_maps, core_ids=list(range(N_CORES)), trace=trace, **spmd_kwargs
    )
    outs = [r["out"] for r in res.results]
    full = np.empty((B, N, QUERY_DIM), dtype=np.float32)
    for b in range(B):
        full[b] = outs[2 * b] + outs[2 * b + 1]
    return full, res


def kernel(**inputs) -> np.ndarray:
    full, _ = run(inputs, trace=False)
    return full


# revision 6
# speedup vs baseline: 1.1798x; 1.0521x over previous
# Bass/Tile Trainium2 kernel for nn_Attention_48816598286380.
#
# Reference computation (B=4, N=512, M=8192, Hq=512, Ck=256, H=8, D=64):
#   q = x @ Wq;  k,v = split(context @ Wkv);  per-head softmax(q k^T / sqrt(D)) v
#   out = attn_out @ Wo + bo
#
# Sharding: 8 cores = 4 batches x 2 head-groups (4 heads each).  Each core
# computes its batch's attention for its 4 heads plus the partial output
# projection over those heads; the host sums the two partial projections per
# batch (bo is split half/half so the sum carries the full bias).
#
# Numerics: Q/K/V and the exp'd scores E are fp8e4 so the two big matmuls
# run in dual-row fp8 mode (2 k-tiles per pass, 0.5 PE cycles/row):
#   scoresT[m, n] = kT.T @ (q8 + q_residual8): the dual-row second k-tile
#     carries a quantized fp8 residual of qT (k-tile dim stride-0 on kT), so
#     only kT's quantization error survives -> rel err ~1.1e-2.
#   E = exp(s/8 - 2) in fp8e4 (softmax shift keeps E <= ~35 << 240), on two
#     engines in parallel: Act exp (PSUM->SBUF, scale+bias) and gpsimd pow
#     c^(s-16), c = e^(1/8) (vpowf ucode; needs a DVE/Act staging copy of
#     s-16 to SBUF since gpsimd cannot read PSUM).
#   numerT[d, n] + denominator row 64 = v_aug.T @ E with TWO m-tiles per
#     instruction (the dual-row k-tile pair; v rows padded to 68 for the
#     ISA's 16-element dual-fp8 weight stride rule).
# kT/qT/v production and the output projection stay f32r (exact inputs).
#
# Engine budget per core (cost model): exp is 128 x [128,1024] tiles split
# Act/Pool; PSUM->SBUF casts split Act/DVE; PE ~80us under the ~95us
# Act/DVE/Pool balance.  PSUM: 2x2-bank score slots (QK->exp), 2x1-bank
# production slots (kT/v/proj -> cast), 2 accumulator banks.

import numpy as np

B, N, M = 4, 512, 8192
QUERY_DIM, INPUT_DIM = 512, 256
HEADS, DIM_HEAD = 8, 64
ATT_DIM = HEADS * DIM_HEAD  # 512
HPC = 4          # heads per core
N_CORES = 8
DH_PAD = 68      # v row pitch: 64 d + ones + 3 pad (16-elem alignment)
CHUNKS = [(0, 512), (512, 512)] + [(m0, 1024) for m0 in range(1024, M, 1024)]
MCHUNK = 1024
SCALE = DIM_HEAD ** -0.5   # 0.125
EBIAS = -2.0               # softmax shift (in exp-arg units)
SBIAS = EBIAS / SCALE      # shift in score units (-16)

# exp engine split: tile index within a pass -> True = Pool pow path
POOL0 = [(i % 2) == 0 for i in range(64)]            # pass 0: 32 on Pool
POOL1 = [(i % 5) in (0, 2) for i in range(64)]       # pass 1: 26 on Pool
# production-cast engine: index within chunk cycle -> True = Act copy
# (kt half copies and v group copies, pass 0 only)
ACT_COPY = [True, False, True, True, False, True, False, True]
AV_LAG = 2       # pairs of delay between exp and the consuming AV matmul

_CACHE = {}


def _build_nc():
    import concourse.bacc as bacc
    import concourse.bass as bass
    import concourse.mybir as mybir
    import concourse.tile as tile

    f32 = mybir.dt.float32
    f32r = mybir.dt.float32r
    fp8 = mybir.dt.float8e4
    EXP = mybir.ActivationFunctionType.Exp
    CPY = mybir.ActivationFunctionType.Copy
    POW = mybir.AluOpType.pow
    DR = mybir.MatmulPerfMode.DoubleRow

    def bc0(ap, ins_dims):
        """Insert stride-0 free dims into an AP after the partition dim."""
        return bass.AP(
            tensor=ap.tensor,
            offset=ap.offset,
            ap=[ap.ap[0]] + [[0, n] for n in ins_dims] + list(ap.ap[1:]),
        )

    nc = bacc.Bacc(None, target_bir_lowering=False)

    ct = nc.dram_tensor("ct", [INPUT_DIM, M], f32r, kind="ExternalInput")
    xt = nc.dram_tensor("xt", [QUERY_DIM, N], f32r, kind="ExternalInput")
    wq = nc.dram_tensor("wq", [QUERY_DIM, HPC * DIM_HEAD], f32r, kind="ExternalInput")
    wk = nc.dram_tensor("wk", [INPUT_DIM, HPC * DIM_HEAD], f32r, kind="ExternalInput")
    wv = nc.dram_tensor("wv", [INPUT_DIM, HPC * DIM_HEAD], f32r, kind="ExternalInput")
    wo = nc.dram_tensor("wo", [DIM_HEAD, HPC, QUERY_DIM], f32r, kind="ExternalInput")
    bobc = nc.dram_tensor("bobc", [128, QUERY_DIM], f32, kind="ExternalInput")
    out = nc.dram_tensor("out", [N, QUERY_DIM], f32, kind="ExternalOutput")

    ct_r = ct[:, :].rearrange("(t p) m -> p t m", p=128)
    xt_r = xt[:, :].rearrange("(t p) n -> p t n", p=128)
    wq_r = wq[:, :].rearrange("(t p) d -> p t d", p=128)
    wk_r = wk[:, :].rearrange("(t p) d -> p t d", p=128)
    wv_r = wv[:, :].rearrange("(t p) d -> p t d", p=128)
    out_r = out[:, :].rearrange("(t p) f -> p t f", p=128)

    with tile.TileContext(nc) as tc:
        with (
            tc.tile_pool(name="const", bufs=1) as cp,
            tc.tile_pool(name="ctp", bufs=2) as ctp,
            tc.tile_pool(name="ktp", bufs=2) as ktp,
            tc.tile_pool(name="ep", bufs=6) as ep,
            tc.tile_pool(name="stp", bufs=4) as stp,
            tc.tile_pool(name="scp", bufs=2, space="PSUM") as scp,
            tc.tile_pool(name="prp", bufs=2, space="PSUM") as prp,
            tc.tile_pool(name="accp", bufs=1, space="PSUM") as accp,
        ):
            # ---- constants ----
            xt_sb = cp.tile([128, 4, N], f32r)
            wq_sb = cp.tile([128, 4, HPC * DIM_HEAD], f32r)
            wk_sb = cp.tile([128, 2, HPC * DIM_HEAD], f32r)
            wv_sb = cp.tile([128, 2, HPC * DIM_HEAD], f32r)
            wo_sb = cp.tile([DIM_HEAD, HPC, QUERY_DIM], f32r)
            bo_bc = cp.tile([128, QUERY_DIM], f32)
            # qT fp8: [d(2 heads x 64), pass, ktile, n]; ktile1 = fp8 residual
            qt8 = cp.tile([128, 2, 2, N], fp8)
            # v fp8: [m-in-tile, m-tile, head, 68]; col 64 ones, 65-67 pad
            v8 = cp.tile([128, M // 128, HPC, DH_PAD], fp8)
            kt8_f1 = ktp.tile([128, M], fp8, tag="ktf1", name="ktf1", bufs=1)
            stack_sb = cp.tile([DIM_HEAD, HPC, N], f32r)
            recip_sb = cp.tile([128, 2, N], f32)
            bcast_sb = cp.tile([DIM_HEAD, 2, N], f32)
            ones64_sb = cp.tile([128, DIM_HEAD], f32)
            cexp_sb = cp.tile([128, 1], f32)
            ebias_sb = cp.tile([128, 1], f32)
            out0_sb = cp.tile([128, 4, QUERY_DIM], f32)
            out_sb = cp.tile([128, 4, QUERY_DIM], f32)

            nc.sync.dma_start(out=wq_sb[:], in_=wq_r)
            nc.sync.dma_start(out=xt_sb[:], in_=xt_r)
            nc.sync.dma_start(out=wk_sb[:], in_=wk_r)

            # PE warm-up (p-state ramp: ~3.4us of activity -> full clock)
            warm_sb = cp.tile([128, 64], f32)
            nc.vector.memset(warm_sb[:], 0.0)
            warm_ps = accp.tile([128, N], f32, tag="acc0", name="warm_ps")
            for w in range(24):
                nc.tensor.matmul(
                    warm_ps[0:64, 0:64], lhsT=warm_sb[:], rhs=warm_sb[:],
                    start=True, stop=True, skip_group_check=True,
                )

            kt_of = {}

            def cast_copy(use_act, dst, src):
                if use_act:
                    nc.scalar.activation(dst, src, CPY, scale=1.0, bias=0.0)
                else:
                    nc.vector.tensor_copy(dst, src)

            def produce_chunk(mc):
                """DMA chunk mc of contextT; return production emitters at
                [128,512] granularity (one prp PSUM slot each)."""
                m0, mlen = CHUNKS[mc]
                ct_t = ctp.tile([128, 2, MCHUNK], f32r, tag="ct", name=f"ct{mc}")
                ct_dma = nc.sync.dma_start(
                    out=ct_t[:, :, 0:mlen], in_=ct_r[:, :, m0:m0 + mlen]
                )
                if mc >= 1:
                    for d in late_dmas:
                        tile.add_dep_helper(ct_dma.ins, d.ins, sync=False,
                                            reason="prologue before ct stream")
                kt_t = ktp.tile([128, MCHUNK], fp8, tag="kt", name=f"kt{mc}")
                for mi in range(m0 // 128, (m0 + mlen) // 128):
                    kt_of[mi] = (kt_t, mi * 128 - m0)
                halves = mlen // 512

                def kt_half(pp, h2, eng):
                    def go():
                        kt_ps = prp.tile([128, 512], f32, tag="pr",
                                         name=f"ktps{pp}{mc}{h2}")
                        for t in range(2):
                            nc.tensor.matmul(
                                kt_ps[:, :],
                                lhsT=wk_sb[:, t, pp * 128:(pp + 1) * 128],
                                rhs=ct_t[:, t, h2 * 512:(h2 + 1) * 512],
                                start=(t == 0), stop=(t == 1),
                                skip_group_check=True,
                            )
                        o = h2 * 512
                        dst = (kt_t[:, o:o + 512] if pp == 0 else
                               kt8_f1[:, m0 + o:m0 + o + 512])
                        cast_copy(eng, dst, kt_ps[:, :])
                    return go

                def v_pair(s2, eng):
                    def go():
                        v_ps = prp.tile([128, 512], f32, tag="pr",
                                        name=f"vps{mc}{s2}")
                        for q in range(2):
                            s = s2 * 2 + q
                            for t in range(2):
                                nc.tensor.matmul(
                                    v_ps[:, q * 256:(q + 1) * 256],
                                    lhsT=ct_t[:, t, s * 128:(s + 1) * 128],
                                    rhs=wv_sb[:, t, :],
                                    start=(t == 0), stop=(t == 1),
                                    skip_group_check=True,
                                )
                        mt0 = m0 // 128 + s2 * 2
                        cast_copy(
                            eng,
                            v8[:, mt0:mt0 + 2, :, 0:DIM_HEAD],
                            v_ps[:].rearrange("p (s h d) -> p s h d", s=2,
                                              h=HPC),
                        )
                    return go

                groups = []
                ci = 0
                for h2 in range(halves):
                    groups.append(kt_half(0, h2, ACT_COPY[ci % 8])); ci += 1
                for s2 in range(halves * 2):
                    groups.append(v_pair(s2, ACT_COPY[ci % 8])); ci += 1
                for h2 in range(halves):
                    groups.append(kt_half(1, h2, ACT_COPY[ci % 8])); ci += 1
                return groups

            def qk(p, mi, sc):
                if p == 0:
                    ks, off = kt_of[mi]
                else:
                    ks, off = kt8_f1, mi * 128
                for h2 in range(2):
                    lhs = ks[h2 * 64:(h2 + 1) * 64, off:off + 128]
                    nc.tensor.matmul(
                        sc[:, h2 * 512:(h2 + 1) * 512],
                        lhsT=bc0(lhs, [2]),
                        rhs=qt8[h2 * 64:(h2 + 1) * 64, p, :, :],
                        start=True, stop=True, perf_mode=DR,
                    )

            def exp_tile(sc, e_t, t, use_pool):
                dst = e_t[:, :, t, :]
                if use_pool:
                    st = stp.tile([128, 2, 512], f32, tag="st", name=f"st{t}")
                    nc.vector.tensor_scalar_add(
                        st[:, :, :], sc[:].rearrange("p (h n) -> p h n", h=2),
                        SBIAS)
                    nc.gpsimd.tensor_tensor(
                        dst, bc0(cexp_sb[:, :1], [2, 512]).opt({0}),
                        st[:, :, :], POW)
                else:
                    nc.scalar.activation(
                        dst, sc[:].rearrange("p (h n) -> p h n", h=2),
                        EXP, scale=SCALE, bias=ebias_sb[:, :])

            def av(p, pair, e_t, acc):
                for h2 in range(2):
                    nc.tensor.matmul(
                        acc[h2][0:DH_PAD, :],
                        lhsT=v8[:, 2 * pair:2 * pair + 2, 2 * p + h2, :],
                        rhs=e_t[:, h2, :, :],
                        start=(pair == 0), stop=(pair == M // 256 - 1),
                        perf_mode=DR, skip_group_check=True,
                    )

            def pass_tail(p, acc):
                bc_ps = scp.tile([128, 1024], f32, tag="sc", name=f"bc{p}")
                for h2 in range(2):
                    nc.vector.reciprocal(
                        recip_sb[DIM_HEAD:DIM_HEAD + 1, h2, :],
                        acc[h2][DIM_HEAD:DIM_HEAD + 1, :],
                    )
                    nc.tensor.matmul(
                        bc_ps[0:DIM_HEAD, h2 * 512:(h2 + 1) * 512],
                        lhsT=ones64_sb[DIM_HEAD:DIM_HEAD + 1, :],
                        rhs=recip_sb[DIM_HEAD:DIM_HEAD + 1, h2, :],
                        start=True, stop=True, skip_group_check=True,
                    )
                    nc.vector.tensor_copy(
                        bcast_sb[:, h2, :],
                        bc_ps[0:DIM_HEAD, h2 * 512:(h2 + 1) * 512],
                    )
                    nc.vector.tensor_mul(
                        stack_sb[:, 2 * p + h2, :], acc[h2][0:DIM_HEAD, :],
                        bcast_sb[:, h2, :]
                    )

            chunk0 = produce_chunk(0)

            late_dmas = []
            late_dmas.append(nc.sync.dma_start(out=wv_sb[:], in_=wv_r))
            late_dmas.append(nc.sync.dma_start(out=wo_sb[:], in_=wo[:, :, :]))
            late_dmas.append(nc.sync.dma_start(out=bo_bc[:], in_=bobc[:, :]))
            nc.vector.memset(ones64_sb[:], 1.0)
            nc.vector.memset(cexp_sb[:], float(np.exp(SCALE)))
            nc.vector.memset(ebias_sb[:], EBIAS)
            nc.gpsimd.memset(v8[:, :, :, DIM_HEAD:DH_PAD], 1.0)

            # qT: rows 0-63 head 2p, 64-127 head 2p+1; fp8 + fp8 residual
            q_ps = scp.tile([128, 1024], f32, tag="sc", name="q_ps")
            for p in range(2):
                for t in range(4):
                    nc.tensor.matmul(
                        q_ps[:, p * 512:(p + 1) * 512],
                        lhsT=wq_sb[:, t, p * 128:(p + 1) * 128],
                        rhs=xt_sb[:, t, :],
                        start=(t == 0), stop=(t == 3),
                        skip_group_check=True,
                    )
            q_r = q_ps[:].rearrange("p (a n) -> p a n", a=2)
            nc.vector.tensor_copy(qt8[:, :, 0, :], q_r)
            nc.vector.tensor_sub(qt8[:, :, 1, :], q_r, qt8[:, :, 0, :])

            # ---- pass 0 (heads 0,1), production interleaved ----
            acc0 = [accp.tile([128, N], f32, tag=f"acc{h2}", name=f"a0{h2}")
                    for h2 in range(2)]

            def attention_tile(p, mi, acc, e_state, pool_map):
                """QK + exp for m-tile mi; AV is emitted AV_LAG pairs late so
                the in-order PE queue never blocks on a still-running exp."""
                pair, t = mi // 2, mi % 2
                sc = scp.tile([128, 1024], f32, tag="sc", name=f"sc{p}{mi}")
                qk(p, mi, sc)
                if t == 0:
                    e_state["tile"] = ep.tile([128, 2, 2, 512], fp8, tag="e",
                                              name=f"e{p}{pair}")
                exp_tile(sc, e_state["tile"], t, pool_map[mi])
                if t == 1:
                    e_state.setdefault("pending", []).append(
                        (pair, e_state["tile"]))
                    if len(e_state["pending"]) > AV_LAG:
                        pj, ej = e_state["pending"].pop(0)
                        av(p, pj, ej, acc)

            def flush_av(p, acc, e_state):
                for pj, ej in e_state.get("pending", []):
                    av(p, pj, ej, acc)
                e_state["pending"] = []

            e_state = {}
            for step in range(len(CHUNKS) + 1):
                prod = (chunk0 if step == 0 else produce_chunk(step)) \
                    if step < len(CHUNKS) else []
                if step >= 1:
                    pm0, pmlen = CHUNKS[step - 1]
                    atts = list(range(pm0 // 128, (pm0 + pmlen) // 128))
                else:
                    atts = []
                n = max(len(prod), len(atts))
                for i in range(n):
                    if i < len(atts):
                        attention_tile(0, atts[i], acc0, e_state, POOL0)
                    if i < len(prod):
                        prod[i]()
            flush_av(0, acc0, e_state)
            pass_tail(0, acc0)

            # partial projection for pair 0 (+ bias) overlaps pass 1;
            # uses the production PSUM slots (free during pass 1)
            def proj(hs, dst_sb, add_sb):
                for nt in range(4):
                    pr = prp.tile([128, 512], f32, tag="pr", name=f"pj{hs}{nt}")
                    for h in range(hs, hs + 2):
                        nc.tensor.matmul(
                            pr[:, :],
                            lhsT=stack_sb[:, h, nt * 128:(nt + 1) * 128],
                            rhs=wo_sb[:, h, :],
                            start=(h == hs), stop=(h == hs + 1),
                            skip_group_check=True,
                        )
                    nc.vector.tensor_add(dst_sb[:, nt, :], pr[:, :], add_sb
                                         if add_sb is not None
                                         else bo_bc[:])

            def proj_pair0():
                for nt in range(4):
                    pr = prp.tile([128, 512], f32, tag="pr", name=f"pj0{nt}")
                    for h in range(2):
                        nc.tensor.matmul(
                            pr[:, :],
                            lhsT=stack_sb[:, h, nt * 128:(nt + 1) * 128],
                            rhs=wo_sb[:, h, :],
                            start=(h == 0), stop=(h == 1),
                            skip_group_check=True,
                        )
                    nc.vector.tensor_add(out0_sb[:, nt, :], pr[:, :], bo_bc[:])

            # ---- pass 1 (heads 2,3): pure attention from resident kT/v ----
            acc1 = [accp.tile([128, N], f32, tag=f"acc{h2}", name=f"a1{h2}")
                    for h2 in range(2)]
            e_state1 = {}
            for mi in range(M // 128):
                attention_tile(1, mi, acc1, e_state1, POOL1)
                if mi == 8:
                    proj_pair0()
            flush_av(1, acc1, e_state1)
            pass_tail(1, acc1)

            # ---- pair-1 projection + combine + store ----
            for nt in range(4):
                pr = prp.tile([128, 512], f32, tag="pr", name=f"pj1{nt}")
                for h in range(2, 4):
                    nc.tensor.matmul(
                        pr[:, :],
                        lhsT=stack_sb[:, h, nt * 128:(nt + 1) * 128],
                        rhs=wo_sb[:, h, :],
                        start=(h == 2), stop=(h == 3),
                        skip_group_check=True,
                    )
                nc.vector.tensor_add(out_sb[:, nt, :], pr[:, :],
                                     out0_sb[:, nt, :])
                nc.sync.dma_start(out=out_r[:, nt, :], in_=out_sb[:, nt, :])

    nc.compile()
    return nc


def _get_nc():
    if "nc" not in _CACHE:
        _CACHE["nc"] = _build_nc()
    return _CACHE["nc"]


def _make_in_maps(x, context, Wq, Wkv, Wo, bo):
    x = np.asarray(x, dtype=np.float32)
    context = np.asarray(context, dtype=np.float32)
    Wq = np.asarray(Wq, dtype=np.float32)
    Wkv = np.asarray(Wkv, dtype=np.float32)
    Wo = np.asarray(Wo, dtype=np.float32)
    bo = np.asarray(bo, dtype=np.float32)

    Wk = Wkv[:, :ATT_DIM]
    Wv = Wkv[:, ATT_DIM:]
    bobc = np.ascontiguousarray(
        np.broadcast_to((bo / 2.0)[None, :], (128, QUERY_DIM)))

    in_maps = []
    for c in range(N_CORES):
        b, g = divmod(c, 2)
        hs = g * HPC * DIM_HEAD
        he = hs + HPC * DIM_HEAD
        wo_core = Wo[hs:he, :].reshape(HPC, DIM_HEAD, QUERY_DIM)
        in_maps.append({
            "ct": np.ascontiguousarray(context[b].T),
            "xt": np.ascontiguousarray(x[b].T),
            "wq": np.ascontiguousarray(Wq[:, hs:he]),
            "wk": np.ascontiguousarray(Wk[:, hs:he]),
            "wv": np.ascontiguousarray(Wv[:, hs:he]),
            "wo": np.ascontiguousarray(wo_core.transpose(1, 0, 2)),
            "bobc": bobc,
        })
    return in_maps


def run(inputs, trace=False, **spmd_kwargs):
    """Run the kernel; returns (full_output [B,N,QUERY_DIM], BassKernelResults)."""
    from concourse.bass_utils import run_bass_kernel_spmd

    nc = _get_nc()
    in_maps = _make_in_maps(**inputs)
    res = run_bass_kernel_spmd(
        nc, in_maps, core_ids=list(range(N_CORES)), trace=trace, **spmd_kwargs
    )
    outs = [r["out"] for r in res.results]
    full = np.empty((B, N, QUERY_DIM), dtype=np.float32)
    for b in range(B):
        full[b] = outs[2 * b] + outs[2 * b + 1]
    return full, res


def kernel(**inputs) -> np.ndarray:
    full, _ = run(inputs, trace=False)
    return full


# revision 7
# speedup vs baseline: 1.1814x; 1.0014x over previous
# Bass/Tile Trainium2 kernel for nn_Attention_48816598286380.
#
# Reference computation (B=4, N=512, M=8192, Hq=512, Ck=256, H=8, D=64):
#   q = x @ Wq;  k,v = split(context @ Wkv);  per-head softmax(q k^T / sqrt(D)) v
#   out = attn_out @ Wo + bo
#
# Sharding: 8 cores = 4 batches x 2 head-groups (4 heads each).  Each core
# computes its batch's attention for its 4 heads plus the partial output
# projection over those heads; the host sums the two partial projections per
# batch (bo is split half/half so the sum carries the full bias).
#
# Numerics: Q/K/V and the exp'd scores E are fp8e4 so the two big matmuls
# run in dual-row fp8 mode (2 k-tiles per pass, 0.5 PE cycles/row):
#   scoresT[m, n] = kT.T @ (q8 + q_residual8): the dual-row second k-tile
#     carries a quantized fp8 residual of qT (k-tile dim stride-0 on kT), so
#     only kT's quantization error survives -> rel err ~1.1e-2.
#   E = exp(s/8 - 2) in fp8e4 (softmax shift keeps E <= ~35 << 240), on two
#     engines in parallel: Act exp (PSUM->SBUF, scale+bias) and gpsimd pow
#     c^(s-16), c = e^(1/8) (vpowf ucode; needs a DVE/Act staging copy of
#     s-16 to SBUF since gpsimd cannot read PSUM).
#   numerT[d, n] + denominator row 64 = v_aug.T @ E with TWO m-tiles per
#     instruction (the dual-row k-tile pair; v rows padded to 68 for the
#     ISA's 16-element dual-fp8 weight stride rule).
# kT/qT/v production and the output projection stay f32r (exact inputs).
#
# Engine budget per core (cost model): exp is 128 x [128,1024] tiles split
# Act/Pool; PSUM->SBUF casts split Act/DVE; PE ~80us under the ~95us
# Act/DVE/Pool balance.  PSUM: 2x2-bank score slots (QK->exp), 2x1-bank
# production slots (kT/v/proj -> cast), 2 accumulator banks.

import numpy as np

B, N, M = 4, 512, 8192
QUERY_DIM, INPUT_DIM = 512, 256
HEADS, DIM_HEAD = 8, 64
ATT_DIM = HEADS * DIM_HEAD  # 512
HPC = 4          # heads per core
N_CORES = 8
DH_PAD = 68      # v row pitch: 64 d + ones + 3 pad (16-elem alignment)
CHUNKS = [(0, 512), (512, 512)] + [(m0, 1024) for m0 in range(1024, M, 1024)]
MCHUNK = 1024
SCALE = DIM_HEAD ** -0.5   # 0.125
EBIAS = -2.0               # softmax shift (in exp-arg units)
SBIAS = EBIAS / SCALE      # shift in score units (-16)

# exp engine split: tile index within a pass -> True = Pool pow path
POOL0 = [(i % 2) == 0 for i in range(64)]            # pass 0: 32 on Pool
POOL1 = [(i % 5) in (0, 2) for i in range(64)]       # pass 1: 26 on Pool
# production-cast engine: index within chunk cycle -> True = Act copy
# (kt half copies and v group copies, pass 0 only)
ACT_COPY = [True, False, True, True, False, True, False, True]
AV_LAG = 3       # pairs of delay between exp and the consuming AV matmul

_CACHE = {}


def _build_nc():
    import concourse.bacc as bacc
    import concourse.bass as bass
    import concourse.mybir as mybir
    import concourse.tile as tile

    f32 = mybir.dt.float32
    f32r = mybir.dt.float32r
    fp8 = mybir.dt.float8e4
    EXP = mybir.ActivationFunctionType.Exp
    CPY = mybir.ActivationFunctionType.Copy
    POW = mybir.AluOpType.pow
    DR = mybir.MatmulPerfMode.DoubleRow

    def bc0(ap, ins_dims):
        """Insert stride-0 free dims into an AP after the partition dim."""
        return bass.AP(
            tensor=ap.tensor,
            offset=ap.offset,
            ap=[ap.ap[0]] + [[0, n] for n in ins_dims] + list(ap.ap[1:]),
        )

    nc = bacc.Bacc(None, target_bir_lowering=False)

    ct = nc.dram_tensor("ct", [INPUT_DIM, M], f32r, kind="ExternalInput")
    xt = nc.dram_tensor("xt", [QUERY_DIM, N], f32r, kind="ExternalInput")
    wq = nc.dram_tensor("wq", [QUERY_DIM, HPC * DIM_HEAD], f32r, kind="ExternalInput")
    wk = nc.dram_tensor("wk", [INPUT_DIM, HPC * DIM_HEAD], f32r, kind="ExternalInput")
    wv = nc.dram_tensor("wv", [INPUT_DIM, HPC * DIM_HEAD], f32r, kind="ExternalInput")
    wo = nc.dram_tensor("wo", [DIM_HEAD, HPC, QUERY_DIM], f32r, kind="ExternalInput")
    bobc = nc.dram_tensor("bobc", [128, QUERY_DIM], f32, kind="ExternalInput")
    out = nc.dram_tensor("out", [N, QUERY_DIM], f32, kind="ExternalOutput")

    ct_r = ct[:, :].rearrange("(t p) m -> p t m", p=128)
    xt_r = xt[:, :].rearrange("(t p) n -> p t n", p=128)
    wq_r = wq[:, :].rearrange("(t p) d -> p t d", p=128)
    wk_r = wk[:, :].rearrange("(t p) d -> p t d", p=128)
    wv_r = wv[:, :].rearrange("(t p) d -> p t d", p=128)
    out_r = out[:, :].rearrange("(t p) f -> p t f", p=128)

    with tile.TileContext(nc) as tc:
        with (
            tc.tile_pool(name="const", bufs=1) as cp,
            tc.tile_pool(name="ctp", bufs=2) as ctp,
            tc.tile_pool(name="ktp", bufs=2) as ktp,
            tc.tile_pool(name="ep", bufs=12) as ep,
            tc.tile_pool(name="stp", bufs=6) as stp,
            tc.tile_pool(name="scp", bufs=2, space="PSUM") as scp,
            tc.tile_pool(name="prp", bufs=2, space="PSUM") as prp,
            tc.tile_pool(name="accp", bufs=1, space="PSUM") as accp,
        ):
            # ---- constants ----
            xt_sb = cp.tile([128, 4, N], f32r)
            wq_sb = cp.tile([128, 4, HPC * DIM_HEAD], f32r)
            wk_sb = cp.tile([128, 2, HPC * DIM_HEAD], f32r)
            wv_sb = cp.tile([128, 2, HPC * DIM_HEAD], f32r)
            wo_sb = cp.tile([DIM_HEAD, HPC, QUERY_DIM], f32r)
            bo_bc = cp.tile([128, QUERY_DIM], f32)
            # qT fp8: [d(2 heads x 64), pass, ktile, n]; ktile1 = fp8 residual
            qt8 = cp.tile([128, 2, 2, N], fp8)
            # v fp8: [m-in-tile, m-tile, head, 68]; col 64 ones, 65-67 pad
            v8 = cp.tile([128, M // 128, HPC, DH_PAD], fp8)
            kt8_f1 = ktp.tile([128, M], fp8, tag="ktf1", name="ktf1", bufs=1)
            stack_sb = cp.tile([DIM_HEAD, HPC, N], f32r)
            recip_sb = cp.tile([128, 2, N], f32)
            bcast_sb = cp.tile([DIM_HEAD, 2, N], f32)
            ones64_sb = cp.tile([128, DIM_HEAD], f32)
            cexp_sb = cp.tile([128, 1], f32)
            ebias_sb = cp.tile([128, 1], f32)
            out0_sb = cp.tile([128, 4, QUERY_DIM], f32)
            out_sb = cp.tile([128, 4, QUERY_DIM], f32)

            nc.sync.dma_start(out=wq_sb[:], in_=wq_r)
            nc.sync.dma_start(out=xt_sb[:], in_=xt_r)
            nc.sync.dma_start(out=wk_sb[:], in_=wk_r)

            # PE warm-up (p-state ramp: ~3.4us of activity -> full clock)
            warm_sb = cp.tile([128, 64], f32)
            nc.vector.memset(warm_sb[:], 0.0)
            warm_ps = accp.tile([128, N], f32, tag="acc0", name="warm_ps")
            for w in range(24):
                nc.tensor.matmul(
                    warm_ps[0:64, 0:64], lhsT=warm_sb[:], rhs=warm_sb[:],
                    start=True, stop=True, skip_group_check=True,
                )

            kt_of = {}

            def cast_copy(use_act, dst, src):
                if use_act:
                    nc.scalar.activation(dst, src, CPY, scale=1.0, bias=0.0)
                else:
                    nc.vector.tensor_copy(dst, src)

            def produce_chunk(mc):
                """DMA chunk mc of contextT; return production emitters at
                [128,512] granularity (one prp PSUM slot each)."""
                m0, mlen = CHUNKS[mc]
                ct_t = ctp.tile([128, 2, MCHUNK], f32r, tag="ct", name=f"ct{mc}")
                ct_dma = nc.sync.dma_start(
                    out=ct_t[:, :, 0:mlen], in_=ct_r[:, :, m0:m0 + mlen]
                )
                if mc >= 1:
                    for d in late_dmas:
                        tile.add_dep_helper(ct_dma.ins, d.ins, sync=False,
                                            reason="prologue before ct stream")
                kt_t = ktp.tile([128, MCHUNK], fp8, tag="kt", name=f"kt{mc}")
                for mi in range(m0 // 128, (m0 + mlen) // 128):
                    kt_of[mi] = (kt_t, mi * 128 - m0)
                halves = mlen // 512

                def kt_half(pp, h2, eng):
                    def go():
                        kt_ps = prp.tile([128, 512], f32, tag="pr",
                                         name=f"ktps{pp}{mc}{h2}")
                        for t in range(2):
                            nc.tensor.matmul(
                                kt_ps[:, :],
                                lhsT=wk_sb[:, t, pp * 128:(pp + 1) * 128],
                                rhs=ct_t[:, t, h2 * 512:(h2 + 1) * 512],
                                start=(t == 0), stop=(t == 1),
                                skip_group_check=True,
                            )
                        o = h2 * 512
                        dst = (kt_t[:, o:o + 512] if pp == 0 else
                               kt8_f1[:, m0 + o:m0 + o + 512])
                        cast_copy(eng, dst, kt_ps[:, :])
                    return go

                def v_pair(s2, eng):
                    def go():
                        v_ps = prp.tile([128, 512], f32, tag="pr",
                                        name=f"vps{mc}{s2}")
                        for q in range(2):
                            s = s2 * 2 + q
                            for t in range(2):
                                nc.tensor.matmul(
                                    v_ps[:, q * 256:(q + 1) * 256],
                                    lhsT=ct_t[:, t, s * 128:(s + 1) * 128],
                                    rhs=wv_sb[:, t, :],
                                    start=(t == 0), stop=(t == 1),
                                    skip_group_check=True,
                                )
                        mt0 = m0 // 128 + s2 * 2
                        cast_copy(
                            eng,
                            v8[:, mt0:mt0 + 2, :, 0:DIM_HEAD],
                            v_ps[:].rearrange("p (s h d) -> p s h d", s=2,
                                              h=HPC),
                        )
                    return go

                groups = []
                ci = 0
                for h2 in range(halves):
                    groups.append(kt_half(0, h2, ACT_COPY[ci % 8])); ci += 1
                for s2 in range(halves * 2):
                    groups.append(v_pair(s2, ACT_COPY[ci % 8])); ci += 1
                for h2 in range(halves):
                    groups.append(kt_half(1, h2, ACT_COPY[ci % 8])); ci += 1
                return groups

            def qk(p, mi, sc):
                if p == 0:
                    ks, off = kt_of[mi]
                else:
                    ks, off = kt8_f1, mi * 128
                for h2 in range(2):
                    lhs = ks[h2 * 64:(h2 + 1) * 64, off:off + 128]
                    nc.tensor.matmul(
                        sc[:, h2 * 512:(h2 + 1) * 512],
                        lhsT=bc0(lhs, [2]),
                        rhs=qt8[h2 * 64:(h2 + 1) * 64, p, :, :],
                        start=True, stop=True, perf_mode=DR,
                    )

            def exp_tile(sc, e_t, t, use_pool):
                dst = e_t[:, :, t, :]
                if use_pool:
                    st = stp.tile([128, 2, 512], f32, tag="st", name=f"st{t}")
                    nc.vector.tensor_scalar_add(
                        st[:, :, :], sc[:].rearrange("p (h n) -> p h n", h=2),
                        SBIAS)
                    nc.gpsimd.tensor_tensor(
                        dst, bc0(cexp_sb[:, :1], [2, 512]).opt({0}),
                        st[:, :, :], POW)
                else:
                    nc.scalar.activation(
                        dst, sc[:].rearrange("p (h n) -> p h n", h=2),
                        EXP, scale=SCALE, bias=ebias_sb[:, :])

            def av(p, pair, e_t, acc):
                for h2 in range(2):
                    nc.tensor.matmul(
                        acc[h2][0:DH_PAD, :],
                        lhsT=v8[:, 2 * pair:2 * pair + 2, 2 * p + h2, :],
                        rhs=e_t[:, h2, :, :],
                        start=(pair == 0), stop=(pair == M // 256 - 1),
                        perf_mode=DR, skip_group_check=True,
                    )

            def pass_tail(p, acc):
                bc_ps = scp.tile([128, 1024], f32, tag="sc", name=f"bc{p}")
                for h2 in range(2):
                    nc.vector.reciprocal(
                        recip_sb[DIM_HEAD:DIM_HEAD + 1, h2, :],
                        acc[h2][DIM_HEAD:DIM_HEAD + 1, :],
                    )
                    nc.tensor.matmul(
                        bc_ps[0:DIM_HEAD, h2 * 512:(h2 + 1) * 512],
                        lhsT=ones64_sb[DIM_HEAD:DIM_HEAD + 1, :],
                        rhs=recip_sb[DIM_HEAD:DIM_HEAD + 1, h2, :],
                        start=True, stop=True, skip_group_check=True,
                    )
                    nc.vector.tensor_copy(
                        bcast_sb[:, h2, :],
                        bc_ps[0:DIM_HEAD, h2 * 512:(h2 + 1) * 512],
                    )
                    nc.vector.tensor_mul(
                        stack_sb[:, 2 * p + h2, :], acc[h2][0:DIM_HEAD, :],
                        bcast_sb[:, h2, :]
                    )

            chunk0 = produce_chunk(0)

            late_dmas = []
            late_dmas.append(nc.sync.dma_start(out=wv_sb[:], in_=wv_r))
            late_dmas.append(nc.sync.dma_start(out=wo_sb[:], in_=wo[:, :, :]))
            late_dmas.append(nc.sync.dma_start(out=bo_bc[:], in_=bobc[:, :]))
            nc.vector.memset(ones64_sb[:], 1.0)
            nc.vector.memset(cexp_sb[:], float(np.exp(SCALE)))
            nc.vector.memset(ebias_sb[:], EBIAS)
            nc.gpsimd.memset(v8[:, :, :, DIM_HEAD:DH_PAD], 1.0)

            # qT: rows 0-63 head 2p, 64-127 head 2p+1; fp8 + fp8 residual
            q_ps = scp.tile([128, 1024], f32, tag="sc", name="q_ps")
            for p in range(2):
                for t in range(4):
                    nc.tensor.matmul(
                        q_ps[:, p * 512:(p + 1) * 512],
                        lhsT=wq_sb[:, t, p * 128:(p + 1) * 128],
                        rhs=xt_sb[:, t, :],
                        start=(t == 0), stop=(t == 3),
                        skip_group_check=True,
                    )
            q_r = q_ps[:].rearrange("p (a n) -> p a n", a=2)
            nc.vector.tensor_copy(qt8[:, :, 0, :], q_r)
            nc.vector.tensor_sub(qt8[:, :, 1, :], q_r, qt8[:, :, 0, :])

            # ---- pass 0 (heads 0,1), production interleaved ----
            acc0 = [accp.tile([128, N], f32, tag=f"acc{h2}", name=f"a0{h2}")
                    for h2 in range(2)]

            def attention_tile(p, mi, acc, e_state, pool_map):
                """QK + exp for m-tile mi; AV is emitted AV_LAG pairs late so
                the in-order PE queue never blocks on a still-running exp."""
                pair, t = mi // 2, mi % 2
                sc = scp.tile([128, 1024], f32, tag="sc", name=f"sc{p}{mi}")
                qk(p, mi, sc)
                if t == 0:
                    e_state["tile"] = ep.tile([128, 2, 2, 512], fp8, tag="e",
                                              name=f"e{p}{pair}")
                exp_tile(sc, e_state["tile"], t, pool_map[mi])
                if t == 1:
                    e_state.setdefault("pending", []).append(
                        (pair, e_state["tile"]))
                    if len(e_state["pending"]) > AV_LAG:
                        pj, ej = e_state["pending"].pop(0)
                        av(p, pj, ej, acc)

            def flush_av(p, acc, e_state):
                for pj, ej in e_state.get("pending", []):
                    av(p, pj, ej, acc)
                e_state["pending"] = []

            e_state = {}
            for step in range(len(CHUNKS) + 1):
                prod = (chunk0 if step == 0 else produce_chunk(step)) \
                    if step < len(CHUNKS) else []
                if step >= 1:
                    pm0, pmlen = CHUNKS[step - 1]
                    atts = list(range(pm0 // 128, (pm0 + pmlen) // 128))
                else:
                    atts = []
                n = max(len(prod), len(atts))
                for i in range(n):
                    if i < len(atts):
                        attention_tile(0, atts[i], acc0, e_state, POOL0)
                    if i < len(prod):
                        prod[i]()
            flush_av(0, acc0, e_state)
            pass_tail(0, acc0)

            # partial projection for pair 0 (+ bias) overlaps pass 1;
            # uses the production PSUM slots (free during pass 1)
            def proj(hs, dst_sb, add_sb):
                for nt in range(4):
                    pr = prp.tile([128, 512], f32, tag="pr", name=f"pj{hs}{nt}")
                    for h in range(hs, hs + 2):
                        nc.tensor.matmul(
                            pr[:, :],
                            lhsT=stack_sb[:, h, nt * 128:(nt + 1) * 128],
                            rhs=wo_sb[:, h, :],
                            start=(h == hs), stop=(h == hs + 1),
                            skip_group_check=True,
                        )
                    nc.vector.tensor_add(dst_sb[:, nt, :], pr[:, :], add_sb
                                         if add_sb is not None
                                         else bo_bc[:])

            def proj_pair0():
                for nt in range(4):
                    pr = prp.tile([128, 512], f32, tag="pr", name=f"pj0{nt}")
                    for h in range(2):
                        nc.tensor.matmul(
                            pr[:, :],
                            lhsT=stack_sb[:, h, nt * 128:(nt + 1) * 128],
                            rhs=wo_sb[:, h, :],
                            start=(h == 0), stop=(h == 1),
                            skip_group_check=True,
                        )
                    nc.vector.tensor_add(out0_sb[:, nt, :], pr[:, :], bo_bc[:])

            # ---- pass 1 (heads 2,3): pure attention from resident kT/v ----
            acc1 = [accp.tile([128, N], f32, tag=f"acc{h2}", name=f"a1{h2}")
                    for h2 in range(2)]
            e_state1 = {}
            for mi in range(M // 128):
                attention_tile(1, mi, acc1, e_state1, POOL1)
                if mi == 8:
                    proj_pair0()
            flush_av(1, acc1, e_state1)
            pass_tail(1, acc1)

            # ---- pair-1 projection + combine + store ----
            for nt in range(4):
                pr = prp.tile([128, 512], f32, tag="pr", name=f"pj1{nt}")
                for h in range(2, 4):
                    nc.tensor.matmul(
                        pr[:, :],
                        lhsT=stack_sb[:, h, nt * 128:(nt + 1) * 128],
                        rhs=wo_sb[:, h, :],
                        start=(h == 2), stop=(h == 3),
                        skip_group_check=True,
                    )
                nc.vector.tensor_add(out_sb[:, nt, :], pr[:, :],
                                     out0_sb[:, nt, :])
                nc.sync.dma_start(out=out_r[:, nt, :], in_=out_sb[:, nt, :])

    nc.compile()
    return nc


def _get_nc():
    if "nc" not in _CACHE:
        _CACHE["nc"] = _build_nc()
    return _CACHE["nc"]


def _make_in_maps(x, context, Wq, Wkv, Wo, bo):
    x = np.asarray(x, dtype=np.float32)
    context = np.asarray(context, dtype=np.float32)
    Wq = np.asarray(Wq, dtype=np.float32)
    Wkv = np.asarray(Wkv, dtype=np.float32)
    Wo = np.asarray(Wo, dtype=np.float32)
    bo = np.asarray(bo, dtype=np.float32)

    Wk = Wkv[:, :ATT_DIM]
    Wv = Wkv[:, ATT_DIM:]
    bobc = np.ascontiguousarray(
        np.broadcast_to((bo / 2.0)[None, :], (128, QUERY_DIM)))

    in_maps = []
    for c in range(N_CORES):
        b, g = divmod(c, 2)
        hs = g * HPC * DIM_HEAD
        he = hs + HPC * DIM_HEAD
        wo_core = Wo[hs:he, :].reshape(HPC, DIM_HEAD, QUERY_DIM)
        in_maps.append({
            "ct": np.ascontiguousarray(context[b].T),
            "xt": np.ascontiguousarray(x[b].T),
            "wq": np.ascontiguousarray(Wq[:, hs:he]),
            "wk": np.ascontiguousarray(Wk[:, hs:he]),
            "wv": np.ascontiguousarray(Wv[:, hs:he]),
            "wo": np.ascontiguousarray(wo_core.transpose(1, 0, 2)),
            "bobc": bobc,
        })
    return in_maps


def run(inputs, trace=False, **spmd_kwargs):
    """Run the kernel; returns (full_output [B,N,QUERY_DIM], BassKernelResults)."""
    from concourse.bass_utils import run_bass_kernel_spmd

    nc = _get_nc()
    in_maps = _make_in_maps(**inputs)
    res = run_bass_kernel_spmd(
        nc, in_maps, core_ids=list(range(N_CORES)), trace=trace, **spmd_kwargs
    )
    outs = [r["out"] for r in res.results]
    full = np.empty((B, N, QUERY_DIM), dtype=np.float32)
    for b in range(B):
        full[b] = outs[2 * b] + outs[2 * b + 1]
    return full, res


def kernel(**inputs) -> np.ndarray:
    full, _ = run(inputs, trace=False)
    return full
